# revision 13
# baseline (speedup 1.0000x reference)
"""Trainium2 Bass kernel for nn_DigitalPhaser (4-stage time-varying allpass
phaser with feedback; x: [64, 240000] f32).

The per-sample recurrence is linear time-varying in an 8-dim state
(s_t = M_t s_{t-1} + c_t x_t, y_t = s_t[6] + x_t) with input-independent
M_t/c_t, so the scan factors into host-precomputed coefficient matrices
and on-device matmuls:

  - time sharded across 8 cores (zero-pad 240000 -> 245760, 30720/core);
    every core keeps all 64 lanes so matmuls get a 64-wide moving operand;
  - NO cross-core communication: the phaser is stable (closed-loop poles
    <= 0.9964), so each core starts W=1920 samples early from a zero
    state and the wrong-initial-state transient decays below 1e-3 before
    the core's real output range begins (verified: rel err 4e-7 vs exact);
  - chunks of L=120 samples; per chunk Y = tril(K) @ X + U @ s_start via
    a [128,128] stationary over the augmented moving [X; s];
  - K/U vary slowly (LFO moves p by ~5e-4/chunk), so phase C linearly
    interpolates between anchor stationaries every 8 chunks:
    Y_j = KU_a @ [X;s]_j + DKU_a @ (alpha_j [X;s]_j), second-order
    accurate (verified 2.1e-3 rel in bf16) -- 4x less K traffic and one
    weight load per 8 chunks;
  - chunk start-states recovered from d_j = G_j X_j (phase A, 4-way
    column-packed tiny matmuls) + a sequential superchunk chain
    T_{q+1} = R_q T_q + Wh_q D_q whose latency hides inside phase A.

Coefficients depend only on the compile-time LFO schedule: computed here
in float64, shipped as per-core kernel inputs.
"""

import os
import numpy as np
import ml_dtypes

import concourse.bass as bass
import concourse.bacc as bacc
import concourse.mybir as mybir
from concourse.tile import TileContext
from concourse.bass_utils import run_bass_kernel_spmd

SAMPLE_RATE = 48000.0
F0 = 0.5
F_MIN = 1000.0
F_MAX = 4000.0
FB = 0.7

B = 64
T = 240000
T_PAD = 245760
N_CORES = 8
T_C = T_PAD // N_CORES     # 30720 payload samples per core
L = 120                    # samples per chunk (contraction 120+8 states)
W = 1920                   # warmup samples (16 chunks) per core
C_P = T_C // L             # 256 payload chunks / core
Q = 16                     # chunks / superchunk
N_SQ = (W + T_C) // L // Q  # 17 superchunks / core (superchunk 0 = warmup)
N_CH_EXT = (W + T_PAD) // L + 1  # 2065 chunk slots (one extra: last anchor)
AI = 8                     # chunks per anchor interval (and per PSUM bank)
NB = C_P // AI             # 32 anchor banks / core
GD = 16                    # chunks per DMA group / SBUF tile (== Q)
ND = C_P // GD             # 16 payload DMA groups
KT = 4                     # anchor banks per K DMA tile
NG = (N_SQ * Q) // 4       # 68 pc-G anchors / core
PACK4 = os.environ.get("BASS_PHASER_PACK4", "1") == "1"

MODE = os.environ.get("BASS_PHASER_MODE", "bf16")  # "f32" | "bf16"


# ---------------------------------------------------------------- host math
def _compute_p(idx):
    t = idx.astype(np.float32) / np.float32(SAMPLE_RATE)
    phase = np.float32(2.0 * np.pi * F0) * t
    frac = np.mod(phase / np.float32(2.0 * np.pi), np.float32(1.0))
    tri = np.where(frac < 0.5, 4.0 * frac - 1.0, 3.0 - 4.0 * frac).astype(np.float32)
    d_min = np.float32(F_MIN * 2.0 / SAMPLE_RATE)
    d_max = np.float32(F_MAX * 2.0 / SAMPLE_RATE)
    depth = np.float32((d_max - d_min) * 0.5)
    lfo = d_min + depth * (np.float32(1.0) + tri)
    tanl = np.tan(lfo.astype(np.float32))
    p = (np.float32(1.0) - tanl) / (np.float32(1.0) + tanl)
    return p.astype(np.float64)


def _build_Mc(p):
    n = p.shape[0]
    M = np.zeros((n, 8, 8))
    c = np.zeros((n, 8))
    r0 = np.zeros((n, 8)); r0[:, 0] = p; r0[:, 1] = -1; r0[:, 6] = p * FB
    c0 = p
    r1 = np.zeros((n, 8)); r1[:, 6] = FB
    c1 = np.ones(n)
    r2 = p[:, None] * r0; r2[:, 2] += p; r2[:, 3] -= 1
    c2 = p * c0
    r4 = p[:, None] * r2; r4[:, 4] += p; r4[:, 5] -= 1
    c4 = p * c2
    r6 = p[:, None] * r4; r6[:, 6] += p; r6[:, 7] -= 1
    c6 = p * c4
    for i, (r, cc) in enumerate([(r0, c0), (r1, c1), (r2, c2), (r0, c0),
                                 (r4, c4), (r2, c2), (r6, c6), (r4, c4)]):
        M[:, i, :] = r
        c[:, i] = cc
    return M, c


def _precompute():
    p64 = _compute_p(np.arange(-W, T_PAD + L, dtype=np.int64))
    M, c = _build_Mc(p64)
    Mb = M.reshape(N_CH_EXT, L, 8, 8)
    cb = c.reshape(N_CH_EXT, L, 8)

    Phi = np.empty((N_CH_EXT, L, 8, 8))
    Phi[:, 0] = Mb[:, 0]
    for r in range(1, L):
        Phi[:, r] = Mb[:, r] @ Phi[:, r - 1]

    K = np.zeros((N_CH_EXT, L, L))
    G = np.zeros((N_CH_EXT, 8, L))
    Tcur = cb.copy()
    for lag in range(L):
        qmax = L - lag
        idx = np.arange(qmax)
        K[:, idx + lag, idx] = Tcur[:, :qmax, 6]
        G[:, :, L - 1 - lag] = Tcur[:, L - 1 - lag, :]
        if lag < L - 1:
            nq = qmax - 1
            Tcur[:, :nq] = np.einsum('nqij,nqj->nqi', Mb[:, lag + 1:], Tcur[:, :nq])
    K[:, np.arange(L), np.arange(L)] += 1.0      # wet-mix identity on the diag

    U = Phi[:, :, 6, :].copy()                   # [N_CH_EXT, L, 8]
    P = Phi[:, L - 1].copy()                     # chunk propagators

    I8 = np.eye(8)
    Wh_all, XiT_all, XiD_all, R_all = [], [], [], []
    for k in range(N_CORES):
        Pq_all = P[k * 256:k * 256 + N_SQ * Q].reshape(N_SQ, Q, 8, 8)
        What = np.zeros((N_SQ, Q, 8, 8))
        Xi_T = np.zeros((N_SQ, Q, 8, 8))
        Xi_D = np.zeros((N_SQ, Q, Q, 8, 8))
        R = np.zeros((N_SQ, 8, 8))
        for q in range(N_SQ):
            Pq = Pq_all[q]
            V = np.zeros((Q, 8, 8)); V[0] = I8
            for m in range(1, Q):
                V[m] = Pq[m - 1] @ V[m - 1]
            Xi_T[q] = V
            for m in range(Q):
                acc = I8
                for mp in range(m - 1, -1, -1):
                    Xi_D[q, m, mp] = acc
                    acc = acc @ Pq[mp]
            acc = I8
            for m in range(Q - 1, -1, -1):
                What[q, m] = acc
                acc = acc @ Pq[m]
            R[q] = acc
        Wh_all.append(What); XiT_all.append(Xi_T)
        XiD_all.append(Xi_D); R_all.append(R)

    return dict(K=K, U=U, G=G, Wh=Wh_all, XiT=XiT_all, XiD=XiD_all, R=R_all)


def _pack_core(coef, k, np_dt):
    base = k * 256
    sl_all = slice(base, base + N_SQ * Q)            # warmup + payload chunks

    # anchor + delta stationaries: anchors at payload chunk 8*b
    anc_idx = base + Q + AI * np.arange(NB + 1)      # [33] ext chunk indices
    KUa = np.zeros((NB + 1, 128, 128))
    KUa[:, 0:L, 0:L] = coef['K'][anc_idx].transpose(0, 2, 1)     # K^T [tau,t]
    KUa[:, L:128, 0:L] = coef['U'][anc_idx].transpose(0, 2, 1)   # U^T [k,t]
    Kt = np.zeros((NB, 128, 256))
    Kt[:, :, 0:128] = KUa[:-1]
    Kt[:, :, 128:256] = KUa[1:] - KUa[:-1]
    Kt = (Kt.reshape(NB // KT, KT, 128, 256).transpose(0, 2, 1, 3)
          .reshape(NB // KT, 128, KT * 256))

    # pc-G anchors: one G per 4 chunks (mid-chunk), [G_a^T] blocks
    g_idx = np.minimum(base + 4 * np.arange(NG) + 2, N_CH_EXT - 1)
    Gt = coef['G'][g_idx].transpose(2, 0, 1).reshape(L, NG * 8)
    Wh = coef['Wh'][k].transpose(1, 3, 0, 2).reshape(Q * 8, N_SQ * 8)
    Rt = coef['R'][k].transpose(0, 2, 1).reshape(N_SQ, 8, 8) \
        .transpose(1, 0, 2).reshape(8, N_SQ * 8)
    # Xi for payload superchunks only (q=1..16)
    XiT = coef['XiT'][k][1:].transpose(3, 0, 1, 2).reshape(8, ND * Q * 8)
    XiD = (coef['XiD'][k][1:].transpose(2, 4, 0, 1, 3)
           .reshape(Q * 8, ND * Q * 8))
    # alpha mask: chunk c (of 16 per group) -> alpha = (c % 8)/8, all 128 rows
    al = np.repeat((np.arange(GD) % AI) / AI, B)[None, :]        # [1, 1024]
    amask = np.broadcast_to(al, (128, GD * B)).copy()
    out = dict(Kt=Kt, Gt=Gt, Wh=Wh, Rt=Rt, XiT=XiT, XiD=XiD, amask=amask)
    return {n: np.ascontiguousarray(a.astype(np_dt)) for n, a in out.items()}


# ---------------------------------------------------------------- device
def _build_nc(mode):
    f32 = mybir.dt.float32
    dt = f32 if mode == "f32" else mybir.dt.bfloat16

    nc = bacc.Bacc(num_devices=N_CORES)
    P_ = lambda name, shape: nc.declare_dram_parameter(name, list(shape), dt,
                                                       isOutput=False)
    xT16 = P_("xT16", (N_SQ, L, GD * B))
    Kt = P_("Kt", (NB // KT, 128, KT * 256))
    Gt = P_("Gt", (L, NG * 8))
    Wh = P_("Wh", (Q * 8, N_SQ * 8))
    Rt = P_("Rt", (8, N_SQ * 8))
    XiT = P_("XiT", (8, ND * Q * 8))
    XiD = P_("XiD", (Q * 8, ND * Q * 8))
    amask = P_("amask", (128, GD * B))
    yT16 = nc.declare_dram_parameter("yT16", [ND, L, GD * B], dt, isOutput=True)

    with TileContext(nc) as tc:
        with (
            tc.tile_pool(name="const", bufs=1) as cp,
            tc.tile_pool(name="xres", bufs=1) as xpool,
            tc.tile_pool(name="kst", bufs=1) as kp,
            tc.tile_pool(name="yst", bufs=4) as yp,
            tc.tile_pool(name="xsp", bufs=2) as xsp,
            tc.tile_pool(name="svp", bufs=2) as svp,
            tc.tile_pool(name="ps_y", bufs=4, space="PSUM") as ps_y,
            tc.tile_pool(name="ps_s", bufs=2, space="PSUM") as ps_s,
            tc.tile_pool(name="ps_a", bufs=2, space="PSUM") as ps_a,
            tc.tile_pool(name="dram", bufs=1, space="DRAM") as dp,
        ):
            # ---- loads.  gpsimd: even x tiles (nothing else -- keeps the
            # queue free); sync: Gt + odd x tiles (+ stacks/injects later);
            # scalar: small consts, anchor stationaries, svd/y stores.
            gt_t = cp.tile([L, NG * 8], dt, tag="gt")
            nc.sync.dma_start(out=gt_t[:], in_=Gt[:, :])
            xg = []
            for g in range(N_SQ):
                t = xpool.tile([128, GD * B], dt, tag=f"x{g}")
                eng = nc.gpsimd if g % 2 == 0 else nc.sync
                eng.dma_start(out=t[0:L, :], in_=xT16[g, :, :])
                xg.append(t)

            def sload(param, shape, tag):
                t = cp.tile(list(shape), dt, tag=tag)
                nc.scalar.dma_start(out=t[:], in_=param[:, :])
                return t

            wh_t = sload(Wh, (Q * 8, N_SQ * 8), "wh")
            rt_t = sload(Rt, (8, N_SQ * 8), "rt")
            xit_t = sload(XiT, (8, ND * Q * 8), "xit")
            xid_t = sload(XiD, (Q * 8, ND * Q * 8), "xid")
            am_t = sload(amask, (128, GD * B), "am")
            kg = []
            for g in range(NB // KT):
                kt = kp.tile([128, KT * 256], dt, tag=f"k{g}")
                nc.scalar.dma_start(out=kt[:], in_=Kt[g, :, :])
                kg.append(kt)

            # ---- phase A (d_j = G_j X_j) + T-chain + Svec, per superchunk
            t_tiles = [None] * (N_SQ + 1)
            for q in range(N_SQ):
                # d for 16 chunks: 4 pc-G anchors, 256 moving cols each
                dsb = svp.tile([8, Q * B], dt, tag="dsb")
                for hh in range(2):
                    pda = ps_a.tile([8, 8 * B], f32, tag="pa")
                    for h2 in range(2):
                        aa = q * 4 + hh * 2 + h2
                        nc.tensor.matmul(
                            pda[:, h2 * 4 * B:(h2 + 1) * 4 * B],
                            gt_t[:, aa * 8:(aa + 1) * 8],
                            xg[q][0:L, (hh * 2 + h2) * 4 * B:
                                  (hh * 2 + h2 + 1) * 4 * B],
                            start=True, stop=True)
                    dst = dsb[:, hh * 8 * B:(hh + 1) * 8 * B]
                    if hh == 0:
                        nc.vector.tensor_copy(out=dst, in_=pda[:])
                    else:
                        nc.scalar.copy(out=dst, in_=pda[:])
                # restack [8, (m b)] -> [(m k), b] via a DRAM bounce (a
                # composite partition dim on an SBUF DMA side mis-lowers,
                # so keep SBUF partition dims simple on both hops)
                dq = cp.tile([Q * 8, B], dt, tag=f"d{q}")
                dqd = dp.tile([Q * 8, B], dt, tag=f"dqd{q}")
                nc.scalar.dma_start(
                    out=dqd[:].rearrange("(m k) b -> k m b", m=Q),
                    in_=dsb[:].rearrange("k (m b) -> k m b", m=Q))
                nc.sync.dma_start(out=dq[:], in_=dqd[:])

                # T_{q+1} = R_q T_q + Wh_q D_q  (superchunk end state)
                if q < N_SQ - 1:
                    pt = ps_s.tile([128, B], f32, tag="ps")
                    if q == 0:
                        nc.tensor.matmul(pt[0:8, :], wh_t[:, 0:8], dq[:],
                                         start=True, stop=True)
                    else:
                        nc.tensor.matmul(pt[0:8, :], rt_t[:, q * 8:(q + 1) * 8],
                                         t_tiles[q][:], start=True, stop=False)
                        nc.tensor.matmul(pt[0:8, :], wh_t[:, q * 8:(q + 1) * 8],
                                         dq[:], start=False, stop=True)
                    tt = cp.tile([8, B], dt, tag=f"t{q + 1}")
                    nc.vector.tensor_copy(out=tt[:], in_=pt[0:8, :])
                    t_tiles[q + 1] = tt

                # Svec_q = XiD_q @ D_q + XiT_q @ T_q ; ship to HBM scratch
                if q >= 1:
                    pv = ps_s.tile([128, B], f32, tag="ps")
                    nc.tensor.matmul(pv[:], xid_t[:, (q - 1) * 128:q * 128],
                                     dq[:], start=True, stop=False)
                    nc.tensor.matmul(pv[:], xit_t[:, (q - 1) * 128:q * 128],
                                     t_tiles[q][:], start=False, stop=True)
                    svs = svp.tile([Q * 8, B], dt, tag="svs")
                    nc.vector.tensor_copy(out=svs[:], in_=pv[:])
                    svd = dp.tile([Q * 8, B], dt, tag=f"svd{q}")
                    nc.scalar.dma_start(out=svd[:], in_=svs[:])
                    # inject states into Xaug rows 120:128
                    nc.sync.dma_start(
                        out=xg[q][L:128, :].rearrange("k (c l) -> k c l", c=GD),
                        in_=svd[:].rearrange("(c k) l -> k c l", c=GD, k=8))

            # ---- phase C: Y = KU_a @ [X;s] + DKU_a @ (alpha*[X;s])
            for g in range(ND):
                xs = xsp.tile([128, GD * B], dt, tag="xs")
                nc.vector.tensor_tensor(out=xs[:], in0=xg[g + 1][:],
                                        in1=am_t[:], op=mybir.AluOpType.mult)
                yt = yp.tile([L, GD * B], dt, tag="y")
                for h in range(2):                    # 2 anchor banks / group
                    b = g * 2 + h
                    kt = kg[b // KT]
                    co = (b % KT) * 256
                    py = ps_y.tile([128, AI * B], f32, tag="py")
                    nc.tensor.matmul(py[:], kt[:, co:co + 128],
                                     xg[g + 1][:, h * AI * B:(h + 1) * AI * B],
                                     start=True, stop=False)
                    nc.tensor.matmul(py[:], kt[:, co + 128:co + 256],
                                     xs[:, h * AI * B:(h + 1) * AI * B],
                                     start=False, stop=True)
                    dst = yt[:, h * AI * B:(h + 1) * AI * B]
                    if (g * 2 + h) % 2 == 0:
                        nc.vector.tensor_copy(out=dst, in_=py[0:L, :])
                    else:
                        nc.scalar.copy(out=dst, in_=py[0:L, :])
                seng = nc.sync if g % 2 == 0 else nc.scalar
                seng.dma_start(out=yT16[g, :, :], in_=yt[:])

    nc.compile()
    return nc


# ---------------------------------------------------------------- driver
_CACHE = {}


def _get_built(mode):
    if mode not in _CACHE:
        coef = _precompute()
        np_dt = np.float32 if mode == "f32" else ml_dtypes.bfloat16
        packed = [_pack_core(coef, k, np_dt) for k in range(N_CORES)]
        nc = _build_nc(mode)
        _CACHE[mode] = (nc, packed, np_dt)
    return _CACHE[mode]


def _run(x, mode, trace=False):
    nc, packed, np_dt = _get_built(mode)
    xp = np.zeros((B, W + T_PAD), np.float32)
    xp[:, W:W + T] = np.asarray(x, dtype=np.float32)
    in_maps = []
    for k in range(N_CORES):
        xc = xp[:, k * T_C:k * T_C + W + T_C].T             # [32640, 64]
        xT16 = (xc.reshape(N_SQ, GD, L, B).transpose(0, 2, 1, 3)
                .reshape(N_SQ, L, GD * B))
        m = dict(packed[k])
        m["xT16"] = np.ascontiguousarray(xT16.astype(np_dt))
        in_maps.append(m)
    res = run_bass_kernel_spmd(nc, in_maps, list(range(N_CORES)), trace=trace)
    y = np.empty((B, T_PAD), np.float32)
    for k in range(N_CORES):
        yT16 = np.asarray(res.results[k]["yT16"]).astype(np.float32)
        yc = yT16.reshape(ND, L, GD, B).transpose(0, 2, 1, 3).reshape(T_C, B)
        y[:, k * T_C:(k + 1) * T_C] = yc.T
    return y[:, :T].astype(np.float32), res


def kernel(x):
    y, _ = _run(x, MODE, trace=False)
    return y


def run_traced(x, mode=MODE):
    return _run(x, mode, trace=True)


# revision 14
# speedup vs baseline: 1.0405x; 1.0405x over previous
"""Trainium2 Bass kernel for nn_DigitalPhaser (4-stage time-varying allpass
phaser with feedback; x: [64, 240000] f32).

The per-sample recurrence is linear time-varying in an 8-dim state
(s_t = M_t s_{t-1} + c_t x_t, y_t = s_t[6] + x_t) with input-independent
M_t/c_t, so the scan factors into host-precomputed coefficient matrices
and on-device matmuls:

  - time sharded across 8 cores (zero-pad 240000 -> 245760, 30720/core);
    every core keeps all 64 lanes so matmuls get a 64-wide moving operand;
  - NO cross-core communication: the phaser is stable (closed-loop poles
    <= 0.9964), so each core starts W=1920 samples early from a zero
    state and the wrong-initial-state transient decays below 1e-3 before
    the core's real output range begins (verified: rel err 4e-7 vs exact);
  - chunks of L=120 samples; per chunk Y = tril(K) @ X + U @ s_start via
    a [128,128] stationary over the augmented moving [X; s];
  - K/U vary slowly (LFO moves p by ~5e-4/chunk), so phase C linearly
    interpolates between anchor stationaries every 8 chunks:
    Y_j = KU_a @ [X;s]_j + DKU_a @ (alpha_j [X;s]_j), second-order
    accurate (verified 2.1e-3 rel in bf16) -- 4x less K traffic and one
    weight load per 8 chunks;
  - chunk start-states recovered from d_j = G_j X_j (phase A, 4-way
    column-packed tiny matmuls) + a sequential superchunk chain
    T_{q+1} = R_q T_q + Wh_q D_q whose latency hides inside phase A.

Coefficients depend only on the compile-time LFO schedule: computed here
in float64, shipped as per-core kernel inputs.
"""

import os
import numpy as np
import ml_dtypes

import concourse.bass as bass
import concourse.bacc as bacc
import concourse.mybir as mybir
from concourse.tile import TileContext
from concourse.bass_utils import run_bass_kernel_spmd

SAMPLE_RATE = 48000.0
F0 = 0.5
F_MIN = 1000.0
F_MAX = 4000.0
FB = 0.7

B = 64
T = 240000
T_PAD = 245760
N_CORES = 8
T_C = T_PAD // N_CORES     # 30720 payload samples per core
L = 120                    # samples per chunk (contraction 120+8 states)
W = 1920                   # warmup samples (16 chunks) per core
C_P = T_C // L             # 256 payload chunks / core
Q = 16                     # chunks / superchunk
N_SQ = (W + T_C) // L // Q  # 17 superchunks / core (superchunk 0 = warmup)
N_CH_EXT = (W + T_PAD) // L + 1  # 2065 chunk slots (one extra: last anchor)
AI = 8                     # chunks per anchor interval (and per PSUM bank)
NB = C_P // AI             # 32 anchor banks / core
GD = 16                    # chunks per DMA group / SBUF tile (== Q)
ND = C_P // GD             # 16 payload DMA groups
KT = 4                     # anchor banks per K DMA tile
NG = (N_SQ * Q) // 4       # 68 pc-G anchors / core
C_LAG = 3                  # phase C emission lag (superchunks)

MODE = os.environ.get("BASS_PHASER_MODE", "bf16")  # "f32" | "bf16"


# ---------------------------------------------------------------- host math
def _compute_p(idx):
    t = idx.astype(np.float32) / np.float32(SAMPLE_RATE)
    phase = np.float32(2.0 * np.pi * F0) * t
    frac = np.mod(phase / np.float32(2.0 * np.pi), np.float32(1.0))
    tri = np.where(frac < 0.5, 4.0 * frac - 1.0, 3.0 - 4.0 * frac).astype(np.float32)
    d_min = np.float32(F_MIN * 2.0 / SAMPLE_RATE)
    d_max = np.float32(F_MAX * 2.0 / SAMPLE_RATE)
    depth = np.float32((d_max - d_min) * 0.5)
    lfo = d_min + depth * (np.float32(1.0) + tri)
    tanl = np.tan(lfo.astype(np.float32))
    p = (np.float32(1.0) - tanl) / (np.float32(1.0) + tanl)
    return p.astype(np.float64)


def _build_Mc(p):
    n = p.shape[0]
    M = np.zeros((n, 8, 8))
    c = np.zeros((n, 8))
    r0 = np.zeros((n, 8)); r0[:, 0] = p; r0[:, 1] = -1; r0[:, 6] = p * FB
    c0 = p
    r1 = np.zeros((n, 8)); r1[:, 6] = FB
    c1 = np.ones(n)
    r2 = p[:, None] * r0; r2[:, 2] += p; r2[:, 3] -= 1
    c2 = p * c0
    r4 = p[:, None] * r2; r4[:, 4] += p; r4[:, 5] -= 1
    c4 = p * c2
    r6 = p[:, None] * r4; r6[:, 6] += p; r6[:, 7] -= 1
    c6 = p * c4
    for i, (r, cc) in enumerate([(r0, c0), (r1, c1), (r2, c2), (r0, c0),
                                 (r4, c4), (r2, c2), (r6, c6), (r4, c4)]):
        M[:, i, :] = r
        c[:, i] = cc
    return M, c


def _precompute():
    p64 = _compute_p(np.arange(-W, T_PAD + L, dtype=np.int64))
    M, c = _build_Mc(p64)
    Mb = M.reshape(N_CH_EXT, L, 8, 8)
    cb = c.reshape(N_CH_EXT, L, 8)

    Phi = np.empty((N_CH_EXT, L, 8, 8))
    Phi[:, 0] = Mb[:, 0]
    for r in range(1, L):
        Phi[:, r] = Mb[:, r] @ Phi[:, r - 1]

    K = np.zeros((N_CH_EXT, L, L))
    G = np.zeros((N_CH_EXT, 8, L))
    Tcur = cb.copy()
    for lag in range(L):
        qmax = L - lag
        idx = np.arange(qmax)
        K[:, idx + lag, idx] = Tcur[:, :qmax, 6]
        G[:, :, L - 1 - lag] = Tcur[:, L - 1 - lag, :]
        if lag < L - 1:
            nq = qmax - 1
            Tcur[:, :nq] = np.einsum('nqij,nqj->nqi', Mb[:, lag + 1:], Tcur[:, :nq])
    K[:, np.arange(L), np.arange(L)] += 1.0      # wet-mix identity on the diag

    U = Phi[:, :, 6, :].copy()                   # [N_CH_EXT, L, 8]
    P = Phi[:, L - 1].copy()                     # chunk propagators

    I8 = np.eye(8)
    Wh_all, XiT_all, XiD_all, R_all = [], [], [], []
    for k in range(N_CORES):
        Pq_all = P[k * 256:k * 256 + N_SQ * Q].reshape(N_SQ, Q, 8, 8)
        What = np.zeros((N_SQ, Q, 8, 8))
        Xi_T = np.zeros((N_SQ, Q, 8, 8))
        Xi_D = np.zeros((N_SQ, Q, Q, 8, 8))
        R = np.zeros((N_SQ, 8, 8))
        for q in range(N_SQ):
            Pq = Pq_all[q]
            V = np.zeros((Q, 8, 8)); V[0] = I8
            for m in range(1, Q):
                V[m] = Pq[m - 1] @ V[m - 1]
            Xi_T[q] = V
            for m in range(Q):
                acc = I8
                for mp in range(m - 1, -1, -1):
                    Xi_D[q, m, mp] = acc
                    acc = acc @ Pq[mp]
            acc = I8
            for m in range(Q - 1, -1, -1):
                What[q, m] = acc
                acc = acc @ Pq[m]
            R[q] = acc
        Wh_all.append(What); XiT_all.append(Xi_T)
        XiD_all.append(Xi_D); R_all.append(R)

    return dict(K=K, U=U, G=G, Wh=Wh_all, XiT=XiT_all, XiD=XiD_all, R=R_all)


def _pack_core(coef, k, np_dt):
    base = k * 256
    sl_all = slice(base, base + N_SQ * Q)            # warmup + payload chunks

    # anchor + delta stationaries: anchors at payload chunk 8*b
    anc_idx = base + Q + AI * np.arange(NB + 1)      # [33] ext chunk indices
    KUa = np.zeros((NB + 1, 128, 128))
    KUa[:, 0:L, 0:L] = coef['K'][anc_idx].transpose(0, 2, 1)     # K^T [tau,t]
    KUa[:, L:128, 0:L] = coef['U'][anc_idx].transpose(0, 2, 1)   # U^T [k,t]
    Kt = np.zeros((NB, 128, 256))
    Kt[:, :, 0:128] = KUa[:-1]
    Kt[:, :, 128:256] = KUa[1:] - KUa[:-1]
    Kt = (Kt.reshape(NB // KT, KT, 128, 256).transpose(0, 2, 1, 3)
          .reshape(NB // KT, 128, KT * 256))

    # pc-G anchors: one G per 4 chunks (mid-chunk), [G_a^T] blocks
    g_idx = np.minimum(base + 4 * np.arange(NG) + 2, N_CH_EXT - 1)
    Gt = coef['G'][g_idx].transpose(2, 0, 1).reshape(L, NG * 8)
    Wh = coef['Wh'][k].transpose(1, 3, 0, 2).reshape(Q * 8, N_SQ * 8)
    Rt = coef['R'][k].transpose(0, 2, 1).reshape(N_SQ, 8, 8) \
        .transpose(1, 0, 2).reshape(8, N_SQ * 8)
    # Xi for payload superchunks only (q=1..16)
    XiT = coef['XiT'][k][1:].transpose(3, 0, 1, 2).reshape(8, ND * Q * 8)
    XiD = (coef['XiD'][k][1:].transpose(2, 4, 0, 1, 3)
           .reshape(Q * 8, ND * Q * 8))
    # alpha mask: chunk c (of 16 per group) -> alpha = (c % 8)/8, all 128 rows
    al = np.repeat((np.arange(GD) % AI) / AI, B)[None, :]        # [1, 1024]
    amask = np.broadcast_to(al, (128, GD * B)).copy()
    out = dict(Kt=Kt, Gt=Gt, Wh=Wh, Rt=Rt, XiT=XiT, XiD=XiD, amask=amask)
    return {n: np.ascontiguousarray(a.astype(np_dt)) for n, a in out.items()}


# ---------------------------------------------------------------- device
def _build_nc(mode):
    f32 = mybir.dt.float32
    dt = f32 if mode == "f32" else mybir.dt.bfloat16

    nc = bacc.Bacc(num_devices=N_CORES)
    P_ = lambda name, shape: nc.declare_dram_parameter(name, list(shape), dt,
                                                       isOutput=False)
    xT16 = P_("xT16", (N_SQ, L, GD * B))
    Kt = P_("Kt", (NB // KT, 128, KT * 256))
    Gt = P_("Gt", (L, NG * 8))
    Wh = P_("Wh", (Q * 8, N_SQ * 8))
    Rt = P_("Rt", (8, N_SQ * 8))
    XiT = P_("XiT", (8, ND * Q * 8))
    XiD = P_("XiD", (Q * 8, ND * Q * 8))
    amask = P_("amask", (128, GD * B))
    yT16 = nc.declare_dram_parameter("yT16", [ND, L, GD * B], dt, isOutput=True)

    with TileContext(nc) as tc:
        with (
            tc.tile_pool(name="const", bufs=1) as cp,
            tc.tile_pool(name="xres", bufs=1) as xpool,
            tc.tile_pool(name="kst", bufs=1) as kp,
            tc.tile_pool(name="yst", bufs=4) as yp,
            tc.tile_pool(name="xsp", bufs=1) as xsp,
            tc.tile_pool(name="svp", bufs=2) as svp,
            tc.tile_pool(name="ps_y", bufs=4, space="PSUM") as ps_y,
            tc.tile_pool(name="ps_s", bufs=2, space="PSUM") as ps_s,
            tc.tile_pool(name="ps_a", bufs=2, space="PSUM") as ps_a,
            tc.tile_pool(name="dram", bufs=1, space="DRAM") as dp,
        ):
            # ---- loads.  gpsimd: even x tiles (nothing else -- keeps the
            # queue free); sync: Gt + odd x tiles (+ stacks/injects later);
            # scalar: small consts, anchor stationaries, svd/y stores.
            gt_t = cp.tile([L, NG * 8], dt, tag="gt")
            nc.sync.dma_start(out=gt_t[:], in_=Gt[:, :])
            xg = []
            for g in range(N_SQ):
                t = xpool.tile([128, GD * B], dt, tag=f"x{g}")
                eng = nc.sync if g % 2 == 0 else nc.scalar
                eng.dma_start(out=t[0:L, :], in_=xT16[g, :, :])
                xg.append(t)

            def sload(param, shape, tag):
                t = cp.tile(list(shape), dt, tag=tag)
                nc.scalar.dma_start(out=t[:], in_=param[:, :])
                return t

            wh_t = sload(Wh, (Q * 8, N_SQ * 8), "wh")
            rt_t = sload(Rt, (8, N_SQ * 8), "rt")
            xit_t = sload(XiT, (8, ND * Q * 8), "xit")
            xid_t = sload(XiD, (Q * 8, ND * Q * 8), "xid")
            am_t = sload(amask, (128, GD * B), "am")
            kg = []
            for g in range(NB // KT):
                kt = kp.tile([128, KT * 256], dt, tag=f"k{g}")
                nc.scalar.dma_start(out=kt[:], in_=Kt[g, :, :])
                kg.append(kt)

            # ---- phase C group emitter: Y = KU_a @ [X;s] + DKU_a @ xs
            xs_tiles = [None] * N_SQ

            def emit_c_group(g):
                xs = xs_tiles[g + 1]
                yt = yp.tile([L, GD * B], dt, tag="y")
                for h in range(2):                    # 2 anchor banks / group
                    bb = g * 2 + h
                    kt = kg[bb // KT]
                    co = (bb % KT) * 256
                    py = ps_y.tile([128, AI * B], f32, tag="py")
                    nc.tensor.matmul(py[:], kt[:, co:co + 128],
                                     xg[g + 1][:, h * AI * B:(h + 1) * AI * B],
                                     start=True, stop=False)
                    nc.tensor.matmul(py[:], kt[:, co + 128:co + 256],
                                     xs[:, h * AI * B:(h + 1) * AI * B],
                                     start=False, stop=True)
                    dst = yt[:, h * AI * B:(h + 1) * AI * B]
                    if bb % 2 == 0:
                        nc.vector.tensor_copy(out=dst, in_=py[0:L, :])
                    else:
                        nc.scalar.copy(out=dst, in_=py[0:L, :])
                seng = nc.sync if g % 2 == 0 else nc.scalar
                seng.dma_start(out=yT16[g, :, :], in_=yt[:])

            # ---- phase A (d_j = G_j X_j) + T-chain + Svec, per superchunk
            t_tiles = [None] * (N_SQ + 1)
            for q in range(N_SQ):
                # d for 16 chunks: 4 pc-G anchors, 256 moving cols each
                dsb = svp.tile([8, Q * B], dt, tag="dsb")
                for hh in range(2):
                    pda = ps_a.tile([8, 8 * B], f32, tag="pa")
                    for h2 in range(2):
                        aa = q * 4 + hh * 2 + h2
                        nc.tensor.matmul(
                            pda[:, h2 * 4 * B:(h2 + 1) * 4 * B],
                            gt_t[:, aa * 8:(aa + 1) * 8],
                            xg[q][0:L, (hh * 2 + h2) * 4 * B:
                                  (hh * 2 + h2 + 1) * 4 * B],
                            start=True, stop=True)
                    dst = dsb[:, hh * 8 * B:(hh + 1) * 8 * B]
                    if hh == 0:
                        nc.vector.tensor_copy(out=dst, in_=pda[:])
                    else:
                        nc.scalar.copy(out=dst, in_=pda[:])
                # restack [8, (m b)] -> [(m k), b] via a DRAM bounce (a
                # composite partition dim on an SBUF DMA side mis-lowers,
                # so keep SBUF partition dims simple on both hops)
                dq = cp.tile([Q * 8, B], dt, tag=f"d{q}")
                dqd = dp.tile([Q * 8, B], dt, tag=f"dqd{q}")
                nc.scalar.dma_start(
                    out=dqd[:].rearrange("(m k) b -> k m b", m=Q),
                    in_=dsb[:].rearrange("k (m b) -> k m b", m=Q))
                nc.sync.dma_start(out=dq[:], in_=dqd[:])

                # T_{q+1} = R_q T_q + Wh_q D_q  (superchunk end state)
                if q < N_SQ - 1:
                    pt = ps_s.tile([128, B], f32, tag="ps")
                    if q == 0:
                        nc.tensor.matmul(pt[0:8, :], wh_t[:, 0:8], dq[:],
                                         start=True, stop=True)
                    else:
                        nc.tensor.matmul(pt[0:8, :], rt_t[:, q * 8:(q + 1) * 8],
                                         t_tiles[q][:], start=True, stop=False)
                        nc.tensor.matmul(pt[0:8, :], wh_t[:, q * 8:(q + 1) * 8],
                                         dq[:], start=False, stop=True)
                    tt = cp.tile([8, B], dt, tag=f"t{q + 1}")
                    nc.vector.tensor_copy(out=tt[:], in_=pt[0:8, :])
                    t_tiles[q + 1] = tt

                # Svec_q = XiD_q @ D_q + XiT_q @ T_q ; ship to HBM scratch
                if q >= 1:
                    pv = ps_s.tile([128, B], f32, tag="ps")
                    nc.tensor.matmul(pv[:], xid_t[:, (q - 1) * 128:q * 128],
                                     dq[:], start=True, stop=False)
                    nc.tensor.matmul(pv[:], xit_t[:, (q - 1) * 128:q * 128],
                                     t_tiles[q][:], start=False, stop=True)
                    svs = svp.tile([Q * 8, B], dt, tag="svs")
                    nc.vector.tensor_copy(out=svs[:], in_=pv[:])
                    svd = dp.tile([Q * 8, B], dt, tag=f"svd{q}")
                    nc.scalar.dma_start(out=svd[:], in_=svs[:])
                    # inject states into Xaug rows 120:128, then the
                    # alpha-scaled copy for the delta stream
                    nc.sync.dma_start(
                        out=xg[q][L:128, :].rearrange("k (c l) -> k c l", c=GD),
                        in_=svd[:].rearrange("(c k) l -> k c l", c=GD, k=8))
                    xs = xsp.tile([128, GD * B], dt, tag=f"xs{q}")
                    nc.vector.tensor_tensor(out=xs[:], in0=xg[q][:],
                                            in1=am_t[:],
                                            op=mybir.AluOpType.mult)
                    xs_tiles[q] = xs

                # phase C interleaved with lag: keeps the PE stream dense
                # (and warm) while phase A is paced by the x-tile DMAs
                if q >= C_LAG:
                    emit_c_group(q - C_LAG)
            for g in range(N_SQ - C_LAG, ND):
                emit_c_group(g)

    nc.compile()
    return nc


# ---------------------------------------------------------------- driver
_CACHE = {}


def _get_built(mode):
    if mode not in _CACHE:
        coef = _precompute()
        np_dt = np.float32 if mode == "f32" else ml_dtypes.bfloat16
        packed = [_pack_core(coef, k, np_dt) for k in range(N_CORES)]
        nc = _build_nc(mode)
        _CACHE[mode] = (nc, packed, np_dt)
    return _CACHE[mode]


def _run(x, mode, trace=False):
    nc, packed, np_dt = _get_built(mode)
    xp = np.zeros((B, W + T_PAD), np.float32)
    xp[:, W:W + T] = np.asarray(x, dtype=np.float32)
    in_maps = []
    for k in range(N_CORES):
        xc = xp[:, k * T_C:k * T_C + W + T_C].T             # [32640, 64]
        xT16 = (xc.reshape(N_SQ, GD, L, B).transpose(0, 2, 1, 3)
                .reshape(N_SQ, L, GD * B))
        m = dict(packed[k])
        m["xT16"] = np.ascontiguousarray(xT16.astype(np_dt))
        in_maps.append(m)
    res = run_bass_kernel_spmd(nc, in_maps, list(range(N_CORES)), trace=trace)
    y = np.empty((B, T_PAD), np.float32)
    for k in range(N_CORES):
        yT16 = np.asarray(res.results[k]["yT16"]).astype(np.float32)
        yc = yT16.reshape(ND, L, GD, B).transpose(0, 2, 1, 3).reshape(T_C, B)
        y[:, k * T_C:(k + 1) * T_C] = yc.T
    return y[:, :T].astype(np.float32), res


def kernel(x):
    y, _ = _run(x, MODE, trace=False)
    return y


def run_traced(x, mode=MODE):
    return _run(x, mode, trace=True)


# revision 16
# speedup vs baseline: 1.0486x; 1.0078x over previous
"""Trainium2 Bass kernel for nn_DigitalPhaser (4-stage time-varying allpass
phaser with feedback; x: [64, 240000] f32).

The per-sample recurrence is linear time-varying in an 8-dim state
(s_t = M_t s_{t-1} + c_t x_t, y_t = s_t[6] + x_t) with input-independent
M_t/c_t, so the scan factors into host-precomputed coefficient matrices
and on-device matmuls:

  - time sharded across 8 cores (zero-pad 240000 -> 245760, 30720/core);
    every core keeps all 64 lanes so matmuls get a 64-wide moving operand;
  - NO cross-core communication: the phaser is stable (closed-loop poles
    <= 0.9964), so each core starts W=1920 samples early from a zero
    state and the wrong-initial-state transient decays below 1e-3 before
    the core's real output range begins (verified: rel err 4e-7 vs exact);
  - chunks of L=120 samples; per chunk Y = tril(K) @ X + U @ s_start via
    a [128,128] stationary over the augmented moving [X; s];
  - K/U vary slowly (LFO moves p by ~5e-4/chunk), so phase C linearly
    interpolates between anchor stationaries every 8 chunks:
    Y_j = KU_a @ [X;s]_j + DKU_a @ (alpha_j [X;s]_j) -- 4x less K traffic
    and one weight load per 8 chunks; phase A uses piecewise-constant
    G anchors every 4 chunks (verified 5.0e-3 rel end-to-end in bf16);
  - chunk start-states recovered hierarchically with NO serial cross-
    superchunk dependency: d-vectors restack to superchunk tiles via
    batched DRAM bounces, E_q = Wh_q D_q, then an all-pairs composition
    T_q = sum_{qp<q} LmE[q,qp] E_qp, then per-superchunk
    s = XiD D + XiT T injected into the moving operands' state rows.

Coefficients depend only on the compile-time LFO schedule: computed here
in float64, shipped as per-core kernel inputs.
"""

import os
import numpy as np
import ml_dtypes

import concourse.bass as bass
import concourse.bacc as bacc
import concourse.mybir as mybir
from concourse.tile import TileContext
from concourse.bass_utils import run_bass_kernel_spmd

SAMPLE_RATE = 48000.0
F0 = 0.5
F_MIN = 1000.0
F_MAX = 4000.0
FB = 0.7

B = 64
T = 240000
T_PAD = 245760
N_CORES = 8
T_C = T_PAD // N_CORES     # 30720 payload samples per core
L = 120                    # samples per chunk (contraction 120+8 states)
W = 1920                   # warmup samples (16 chunks) per core
C_P = T_C // L             # 256 payload chunks / core
Q = 16                     # chunks / superchunk
N_SQ = (W + T_C) // L // Q  # 17 superchunks / core (superchunk 0 = warmup)
N_CH_EXT = (W + T_PAD) // L + 1  # 2065 chunk slots (one extra: last anchor)
AI = 8                     # chunks per K anchor interval (and PSUM bank)
NB = C_P // AI             # 32 K-anchor banks / core
GD = 16                    # chunks per DMA group / SBUF tile (== Q)
ND = C_P // GD             # 16 payload DMA groups
KT = 4                     # K-anchor banks per K DMA tile
NG = (N_SQ * Q) // 4       # 68 pc-G anchors / core
SB = 4                     # superchunks per restack bounce batch
NBATCH = (N_SQ + SB - 1) // SB  # 5 batches (4,4,4,4,1)

MODE = os.environ.get("BASS_PHASER_MODE", "bf16")  # "f32" | "bf16"


# ---------------------------------------------------------------- host math
def _compute_p(idx):
    t = idx.astype(np.float32) / np.float32(SAMPLE_RATE)
    phase = np.float32(2.0 * np.pi * F0) * t
    frac = np.mod(phase / np.float32(2.0 * np.pi), np.float32(1.0))
    tri = np.where(frac < 0.5, 4.0 * frac - 1.0, 3.0 - 4.0 * frac).astype(np.float32)
    d_min = np.float32(F_MIN * 2.0 / SAMPLE_RATE)
    d_max = np.float32(F_MAX * 2.0 / SAMPLE_RATE)
    depth = np.float32((d_max - d_min) * 0.5)
    lfo = d_min + depth * (np.float32(1.0) + tri)
    tanl = np.tan(lfo.astype(np.float32))
    p = (np.float32(1.0) - tanl) / (np.float32(1.0) + tanl)
    return p.astype(np.float64)


def _build_Mc(p):
    n = p.shape[0]
    M = np.zeros((n, 8, 8))
    c = np.zeros((n, 8))
    r0 = np.zeros((n, 8)); r0[:, 0] = p; r0[:, 1] = -1; r0[:, 6] = p * FB
    c0 = p
    r1 = np.zeros((n, 8)); r1[:, 6] = FB
    c1 = np.ones(n)
    r2 = p[:, None] * r0; r2[:, 2] += p; r2[:, 3] -= 1
    c2 = p * c0
    r4 = p[:, None] * r2; r4[:, 4] += p; r4[:, 5] -= 1
    c4 = p * c2
    r6 = p[:, None] * r4; r6[:, 6] += p; r6[:, 7] -= 1
    c6 = p * c4
    for i, (r, cc) in enumerate([(r0, c0), (r1, c1), (r2, c2), (r0, c0),
                                 (r4, c4), (r2, c2), (r6, c6), (r4, c4)]):
        M[:, i, :] = r
        c[:, i] = cc
    return M, c


def _precompute():
    p64 = _compute_p(np.arange(-W, T_PAD + L, dtype=np.int64))
    M, c = _build_Mc(p64)
    Mb = M.reshape(N_CH_EXT, L, 8, 8)
    cb = c.reshape(N_CH_EXT, L, 8)

    Phi = np.empty((N_CH_EXT, L, 8, 8))
    Phi[:, 0] = Mb[:, 0]
    for r in range(1, L):
        Phi[:, r] = Mb[:, r] @ Phi[:, r - 1]

    K = np.zeros((N_CH_EXT, L, L))
    G = np.zeros((N_CH_EXT, 8, L))
    Tcur = cb.copy()
    for lag in range(L):
        qmax = L - lag
        idx = np.arange(qmax)
        K[:, idx + lag, idx] = Tcur[:, :qmax, 6]
        G[:, :, L - 1 - lag] = Tcur[:, L - 1 - lag, :]
        if lag < L - 1:
            nq = qmax - 1
            Tcur[:, :nq] = np.einsum('nqij,nqj->nqi', Mb[:, lag + 1:], Tcur[:, :nq])
    K[:, np.arange(L), np.arange(L)] += 1.0      # wet-mix identity on the diag

    U = Phi[:, :, 6, :].copy()                   # [N_CH_EXT, L, 8]
    P = Phi[:, L - 1].copy()                     # chunk propagators

    I8 = np.eye(8)
    Wh_all, XiT_all, XiD_all, LmE_all = [], [], [], []
    for k in range(N_CORES):
        Pq_all = P[k * 256:k * 256 + N_SQ * Q].reshape(N_SQ, Q, 8, 8)
        What = np.zeros((N_SQ, Q, 8, 8))
        Xi_T = np.zeros((N_SQ, Q, 8, 8))
        Xi_D = np.zeros((N_SQ, Q, Q, 8, 8))
        R = np.zeros((N_SQ, 8, 8))
        for q in range(N_SQ):
            Pq = Pq_all[q]
            V = np.zeros((Q, 8, 8)); V[0] = I8
            for m in range(1, Q):
                V[m] = Pq[m - 1] @ V[m - 1]
            Xi_T[q] = V
            for m in range(Q):
                acc = I8
                for mp in range(m - 1, -1, -1):
                    Xi_D[q, m, mp] = acc
                    acc = acc @ Pq[mp]
            acc = I8
            for m in range(Q - 1, -1, -1):
                What[q, m] = acc
                acc = acc @ Pq[m]
            R[q] = acc
        # T_q = sum_{qp<q} LmE[q, qp] @ E_qp   (core start state is zero)
        LmE = np.zeros((N_SQ, N_SQ, 8, 8))
        for q in range(1, N_SQ):
            acc = I8
            for qp in range(q - 1, -1, -1):
                LmE[q, qp] = acc
                acc = acc @ R[qp]
        Wh_all.append(What); XiT_all.append(Xi_T)
        XiD_all.append(Xi_D); LmE_all.append(LmE)

    return dict(K=K, U=U, G=G, Wh=Wh_all, XiT=XiT_all, XiD=XiD_all,
                LmE=LmE_all)


def _pack_core(coef, k, np_dt):
    base = k * 256

    # K anchor + delta stationaries: anchors at payload chunk 8*b
    anc_idx = base + Q + AI * np.arange(NB + 1)      # [33] ext chunk indices
    KUa = np.zeros((NB + 1, 128, 128))
    KUa[:, 0:L, 0:L] = coef['K'][anc_idx].transpose(0, 2, 1)     # K^T [tau,t]
    KUa[:, L:128, 0:L] = coef['U'][anc_idx].transpose(0, 2, 1)   # U^T [k,t]
    Kt = np.zeros((NB, 128, 256))
    Kt[:, :, 0:128] = KUa[:-1]
    Kt[:, :, 128:256] = KUa[1:] - KUa[:-1]
    Kt = (Kt.reshape(NB // KT, KT, 128, 256).transpose(0, 2, 1, 3)
          .reshape(NB // KT, 128, KT * 256))

    # pc-G anchors: one G per 4 chunks (mid-chunk), [G_a^T] blocks
    g_idx = np.minimum(base + 4 * np.arange(NG) + 2, N_CH_EXT - 1)
    Gt = coef['G'][g_idx].transpose(2, 0, 1).reshape(L, NG * 8)

    Wh = coef['Wh'][k].transpose(1, 3, 0, 2).reshape(Q * 8, N_SQ * 8)
    # all-pairs superchunk composition, payload superchunks on the out side
    LmE = (coef['LmE'][k][1:].transpose(3, 1, 0, 2)
           .reshape(8, N_SQ * 128))
    # Xi for payload superchunks only (q=1..16)
    XiT = coef['XiT'][k][1:].transpose(3, 0, 1, 2).reshape(8, ND * Q * 8)
    XiD = (coef['XiD'][k][1:].transpose(2, 4, 0, 1, 3)
           .reshape(Q * 8, ND * Q * 8))
    # alpha mask: chunk c (of 16 per group) -> alpha = (c % 8)/8, all rows
    al = np.repeat((np.arange(GD) % AI) / AI, B)[None, :]        # [1, 1024]
    amask = np.broadcast_to(al, (128, GD * B)).copy()
    out = dict(Kt=Kt, Gt=Gt, Wh=Wh, LmE=LmE, XiT=XiT, XiD=XiD, amask=amask)
    return {n: np.ascontiguousarray(a.astype(np_dt)) for n, a in out.items()}


# ---------------------------------------------------------------- device
def _build_nc(mode):
    f32 = mybir.dt.float32
    dt = f32 if mode == "f32" else mybir.dt.bfloat16

    nc = bacc.Bacc(num_devices=N_CORES)
    P_ = lambda name, shape: nc.declare_dram_parameter(name, list(shape), dt,
                                                       isOutput=False)
    xT16 = P_("xT16", (N_SQ, L, GD * B))
    Kt = P_("Kt", (NB // KT, 128, KT * 256))
    Gt = P_("Gt", (L, NG * 8))
    Wh = P_("Wh", (Q * 8, N_SQ * 8))
    LmE = P_("LmE", (8, N_SQ * 128))
    XiT = P_("XiT", (8, ND * Q * 8))
    XiD = P_("XiD", (Q * 8, ND * Q * 8))
    amask = P_("amask", (128, GD * B))
    yT16 = nc.declare_dram_parameter("yT16", [ND, L, GD * B], dt, isOutput=True)

    with TileContext(nc) as tc:
        with (
            tc.tile_pool(name="const", bufs=1) as cp,
            tc.tile_pool(name="xres", bufs=1) as xpool,
            tc.tile_pool(name="kst", bufs=1) as kp,
            tc.tile_pool(name="yst", bufs=4) as yp,
            tc.tile_pool(name="xsp", bufs=1) as xsp,
            tc.tile_pool(name="svp", bufs=2) as svp,
            tc.tile_pool(name="ps_y", bufs=4, space="PSUM") as ps_y,
            tc.tile_pool(name="ps_s", bufs=2, space="PSUM") as ps_s,
            tc.tile_pool(name="ps_a", bufs=2, space="PSUM") as ps_a,
            tc.tile_pool(name="dram", bufs=1, space="DRAM") as dp,
        ):
            # ---- loads. x tiles alternate the two HWDGE rings (sync even /
            # scalar odd), K tiles interleave with the x-odd stream so both
            # arrive in consumption order; small consts go first.
            gt_t = cp.tile([L, NG * 8], dt, tag="gt")
            nc.sync.dma_start(out=gt_t[:], in_=Gt[:, :])

            def sload(param, shape, tag):
                t = cp.tile(list(shape), dt, tag=tag)
                nc.scalar.dma_start(out=t[:], in_=param[:, :])
                return t

            wh_t = sload(Wh, (Q * 8, N_SQ * 8), "wh")
            lme_t = sload(LmE, (8, N_SQ * 128), "lme")
            xit_t = sload(XiT, (8, ND * Q * 8), "xit")
            xid_t = sload(XiD, (Q * 8, ND * Q * 8), "xid")
            am_t = sload(amask, (128, GD * B), "am")

            xg = [None] * N_SQ
            kg = [None] * (NB // KT)
            for g in range(N_SQ):
                t = xpool.tile([128, GD * B], dt, tag=f"x{g}")
                eng = nc.sync if g % 2 == 0 else nc.scalar
                eng.dma_start(out=t[0:L, :], in_=xT16[g, :, :])
                xg[g] = t
                if g % 2 == 1 and (g - 1) // 2 < NB // KT:
                    j = (g - 1) // 2
                    kt = kp.tile([128, KT * 256], dt, tag=f"k{j}")
                    nc.scalar.dma_start(out=kt[:], in_=Kt[j, :, :])
                    kg[j] = kt

            # ---- phase A: d_j = G_a X_j (pc-G anchors, 4 chunks/anchor);
            # batched DRAM-bounce restack to [(m k), b]; E_q = Wh_q D_q
            dq4 = [None] * NBATCH
            e_parts = [None] * N_SQ
            for s in range(NBATCH):
                qs = list(range(s * SB, min((s + 1) * SB, N_SQ)))
                dsb = svp.tile([8, len(qs) * Q * B], dt, tag=f"dsb{s}")
                for qi, q in enumerate(qs):
                    for hh in range(2):
                        pda = ps_a.tile([8, 8 * B], f32, tag="pa")
                        for h2 in range(2):
                            aa = q * 4 + hh * 2 + h2
                            nc.tensor.matmul(
                                pda[:, h2 * 4 * B:(h2 + 1) * 4 * B],
                                gt_t[:, aa * 8:(aa + 1) * 8],
                                xg[q][0:L, (hh * 2 + h2) * 4 * B:
                                      (hh * 2 + h2 + 1) * 4 * B],
                                start=True, stop=True)
                        dst = dsb[:, (qi * 2 + hh) * 8 * B:
                                  (qi * 2 + hh + 1) * 8 * B]
                        if hh == 0:
                            nc.vector.tensor_copy(out=dst, in_=pda[:])
                        else:
                            nc.scalar.copy(out=dst, in_=pda[:])
                # restack [8, (s m b)] -> [(m k), (s b)] via a DRAM bounce
                dqd = dp.tile([Q * 8, len(qs) * B], dt, tag=f"dqd{s}")
                nc.scalar.dma_start(
                    out=dqd[:].rearrange("(m k) (s b) -> k s m b",
                                         m=Q, s=len(qs)),
                    in_=dsb[:].rearrange("k (s m b) -> k s m b",
                                         s=len(qs), m=Q))
                dqt = cp.tile([Q * 8, len(qs) * B], dt, tag=f"dq{s}")
                nc.sync.dma_start(out=dqt[:], in_=dqd[:])
                dq4[s] = dqt
                for qi, q in enumerate(qs):
                    pe = ps_s.tile([128, B], f32, tag="ps")
                    nc.tensor.matmul(pe[0:8, :], wh_t[:, q * 8:(q + 1) * 8],
                                     dqt[:, qi * B:(qi + 1) * B],
                                     start=True, stop=True)
                    ep = cp.tile([8, B], dt, tag=f"e{q}")
                    nc.vector.tensor_copy(out=ep[:], in_=pe[0:8, :])
                    e_parts[q] = ep

            # ---- all-pairs Tvec: T_q = sum_qp LmE[q,qp] E_qp, bounce to
            # [8, q-major] layout for use as matmul moving operands
            ptv = ps_s.tile([128, B], f32, tag="ps")
            for qp in range(N_SQ):
                nc.tensor.matmul(ptv[:], lme_t[:, qp * 128:(qp + 1) * 128],
                                 e_parts[qp][:],
                                 start=(qp == 0), stop=(qp == N_SQ - 1))
            tvs = svp.tile([ND * 8, B], dt, tag="tvs")
            nc.vector.tensor_copy(out=tvs[:], in_=ptv[:])
            tvT_t = cp.tile([8, ND * B], dt, tag="tvT")
            tv_dram = dp.tile([ND * 8, B], dt, tag="tvd")
            nc.scalar.dma_start(out=tv_dram[:], in_=tvs[:])
            nc.sync.dma_start(
                out=tvT_t[:].rearrange("i (q l) -> i q l", q=ND),
                in_=tv_dram[:].rearrange("(q i) l -> i q l", q=ND, i=8))

            # ---- Svec_q = XiD_q @ D_q + XiT_q @ T_q ; inject into state
            # rows; then the alpha-scaled copy for the delta stream
            xs_tiles = [None] * N_SQ
            for q in range(1, N_SQ):
                s, qi = divmod(q, SB)
                pv = ps_s.tile([128, B], f32, tag="ps")
                nc.tensor.matmul(pv[:], xid_t[:, (q - 1) * 128:q * 128],
                                 dq4[s][:, qi * B:(qi + 1) * B],
                                 start=True, stop=False)
                nc.tensor.matmul(pv[:], xit_t[:, (q - 1) * 128:q * 128],
                                 tvT_t[:, (q - 1) * B:q * B],
                                 start=False, stop=True)
                svs = svp.tile([Q * 8, B], dt, tag="svs")
                nc.vector.tensor_copy(out=svs[:], in_=pv[:])
                svd = dp.tile([Q * 8, B], dt, tag=f"svd{q}")
                nc.scalar.dma_start(out=svd[:], in_=svs[:])
                nc.sync.dma_start(
                    out=xg[q][L:128, :].rearrange("k (c l) -> k c l", c=GD),
                    in_=svd[:].rearrange("(c k) l -> k c l", c=GD, k=8))
                xs = xsp.tile([128, GD * B], dt, tag=f"xs{q}")
                eng = nc.vector if q <= 8 else nc.gpsimd
                eng.tensor_tensor(out=xs[:], in0=xg[q][:], in1=am_t[:],
                                  op=mybir.AluOpType.mult)
                xs_tiles[q] = xs

            # ---- phase C: Y = KU_a @ [X;s] + DKU_a @ (alpha*[X;s])
            for g in range(ND):
                xs = xs_tiles[g + 1]
                yt = yp.tile([L, GD * B], dt, tag="y")
                for h in range(2):                    # 2 anchor banks / group
                    bb = g * 2 + h
                    kt = kg[bb // KT]
                    co = (bb % KT) * 256
                    py = ps_y.tile([128, AI * B], f32, tag="py")
                    nc.tensor.matmul(py[:], kt[:, co:co + 128],
                                     xg[g + 1][:, h * AI * B:(h + 1) * AI * B],
                                     start=True, stop=False)
                    nc.tensor.matmul(py[:], kt[:, co + 128:co + 256],
                                     xs[:, h * AI * B:(h + 1) * AI * B],
                                     start=False, stop=True)
                    dst = yt[:, h * AI * B:(h + 1) * AI * B]
                    if bb % 2 == 0:
                        nc.vector.tensor_copy(out=dst, in_=py[0:L, :])
                    else:
                        nc.scalar.copy(out=dst, in_=py[0:L, :])
                seng = nc.sync if g % 2 == 0 else nc.scalar
                seng.dma_start(out=yT16[g, :, :], in_=yt[:])

    nc.compile()
    return nc


# ---------------------------------------------------------------- driver
_CACHE = {}


def _get_built(mode):
    if mode not in _CACHE:
        coef = _precompute()
        np_dt = np.float32 if mode == "f32" else ml_dtypes.bfloat16
        packed = [_pack_core(coef, k, np_dt) for k in range(N_CORES)]
        nc = _build_nc(mode)
        _CACHE[mode] = (nc, packed, np_dt)
    return _CACHE[mode]


def _run(x, mode, trace=False):
    nc, packed, np_dt = _get_built(mode)
    xp = np.zeros((B, W + T_PAD), np.float32)
    xp[:, W:W + T] = np.asarray(x, dtype=np.float32)
    in_maps = []
    for k in range(N_CORES):
        xc = xp[:, k * T_C:k * T_C + W + T_C].T             # [32640, 64]
        xT16 = (xc.reshape(N_SQ, GD, L, B).transpose(0, 2, 1, 3)
                .reshape(N_SQ, L, GD * B))
        m = dict(packed[k])
        m["xT16"] = np.ascontiguousarray(xT16.astype(np_dt))
        in_maps.append(m)
    res = run_bass_kernel_spmd(nc, in_maps, list(range(N_CORES)), trace=trace)
    y = np.empty((B, T_PAD), np.float32)
    for k in range(N_CORES):
        yT16 = np.asarray(res.results[k]["yT16"]).astype(np.float32)
        yc = yT16.reshape(ND, L, GD, B).transpose(0, 2, 1, 3).reshape(T_C, B)
        y[:, k * T_C:(k + 1) * T_C] = yc.T
    return y[:, :T].astype(np.float32), res


def kernel(x):
    y, _ = _run(x, MODE, trace=False)
    return y


def run_traced(x, mode=MODE):
    return _run(x, mode, trace=True)


# revision 20
# speedup vs baseline: 1.1229x; 1.0708x over previous
"""Trainium2 Bass kernel for nn_DigitalPhaser (4-stage time-varying allpass
phaser with feedback; x: [64, 240000] f32).

The per-sample recurrence is linear time-varying in an 8-dim state
(s_t = M_t s_{t-1} + c_t x_t, y_t = s_t[6] + x_t) with input-independent
M_t/c_t, so the scan factors into host-precomputed coefficient matrices
and on-device matmuls:

  - time sharded across 8 cores (zero-pad 240000 -> 245760, 30720/core);
    every core keeps all 64 lanes so matmuls get a 64-wide moving operand;
  - NO cross-core communication: the phaser is stable (closed-loop poles
    <= 0.9964), so each core starts W=1920 samples early from a zero
    state and the wrong-initial-state transient decays below 1e-3 before
    the core's real output range begins (verified: rel err 4e-7 vs exact);
  - chunks of L=120 samples; per chunk Y = tril(K) @ X + U @ s_start via
    a [128,128] stationary over the augmented moving [X; s];
  - K/U vary slowly (LFO moves p by ~5e-4/chunk), so phase C linearly
    interpolates between anchor stationaries every 8 chunks:
    Y_j = KU_a @ [X;s]_j + DKU_a @ (alpha_j [X;s]_j) -- 4x less K traffic
    and one weight load per 8 chunks; phase A uses piecewise-constant
    G anchors every 4 chunks (verified 5.0e-3 rel end-to-end in bf16);
  - chunk start-states recovered hierarchically with NO serial cross-
    superchunk dependency: d-vectors restack to superchunk tiles via
    batched DRAM bounces, E_q = Wh_q D_q, then an all-pairs composition
    T_q = sum_{qp<q} LmE[q,qp] E_qp, then per-superchunk
    s = XiD D + XiT T injected into the moving operands' state rows.

Coefficients depend only on the compile-time LFO schedule: computed here
in float64, shipped as per-core kernel inputs.
"""

import os
import numpy as np
import ml_dtypes

import concourse.bass as bass
import concourse.bacc as bacc
import concourse.mybir as mybir
from concourse.tile import TileContext
from concourse.bass_utils import run_bass_kernel_spmd

SAMPLE_RATE = 48000.0
F0 = 0.5
F_MIN = 1000.0
F_MAX = 4000.0
FB = 0.7

B = 64
T = 240000
T_PAD = 245760
N_CORES = 8
T_C = T_PAD // N_CORES     # 30720 payload samples per core
L = 120                    # samples per chunk (contraction 120+8 states)
W = 1920                   # warmup samples (16 chunks) per core
C_P = T_C // L             # 256 payload chunks / core
Q = 16                     # chunks / superchunk
N_SQ = (W + T_C) // L // Q  # 17 superchunks / core (superchunk 0 = warmup)
N_CH_EXT = (W + T_PAD) // L + 1  # 2065 chunk slots (one extra: last anchor)
AI = 16                    # chunks per K anchor interval (== group)
NA = C_P // AI             # 16 K anchors / core
GD = 16                    # chunks per DMA group / SBUF tile (== Q)
ND = C_P // GD             # 16 payload DMA groups
KT = 4                     # K anchors per K DMA tile
NG = (N_SQ * Q) // 4       # 68 pc-G anchors / core
SB = 4                     # superchunks per restack bounce batch
NBATCH = (N_SQ + SB - 1) // SB  # 5 batches (4,4,4,4,1)
# within-batch superchunk pairs for state stores (q=1..16)
SV_PAIRS = [(1, 2), (3,), (4, 5), (6, 7), (8, 9), (10, 11), (12, 13),
            (14, 15), (16,)]

MODE = os.environ.get("BASS_PHASER_MODE", "bf16")  # "f32" | "bf16"


# ---------------------------------------------------------------- host math
def _compute_p(idx):
    t = idx.astype(np.float32) / np.float32(SAMPLE_RATE)
    phase = np.float32(2.0 * np.pi * F0) * t
    frac = np.mod(phase / np.float32(2.0 * np.pi), np.float32(1.0))
    tri = np.where(frac < 0.5, 4.0 * frac - 1.0, 3.0 - 4.0 * frac).astype(np.float32)
    d_min = np.float32(F_MIN * 2.0 / SAMPLE_RATE)
    d_max = np.float32(F_MAX * 2.0 / SAMPLE_RATE)
    depth = np.float32((d_max - d_min) * 0.5)
    lfo = d_min + depth * (np.float32(1.0) + tri)
    tanl = np.tan(lfo.astype(np.float32))
    p = (np.float32(1.0) - tanl) / (np.float32(1.0) + tanl)
    return p.astype(np.float64)


def _build_Mc(p):
    n = p.shape[0]
    M = np.zeros((n, 8, 8))
    c = np.zeros((n, 8))
    r0 = np.zeros((n, 8)); r0[:, 0] = p; r0[:, 1] = -1; r0[:, 6] = p * FB
    c0 = p
    r1 = np.zeros((n, 8)); r1[:, 6] = FB
    c1 = np.ones(n)
    r2 = p[:, None] * r0; r2[:, 2] += p; r2[:, 3] -= 1
    c2 = p * c0
    r4 = p[:, None] * r2; r4[:, 4] += p; r4[:, 5] -= 1
    c4 = p * c2
    r6 = p[:, None] * r4; r6[:, 6] += p; r6[:, 7] -= 1
    c6 = p * c4
    for i, (r, cc) in enumerate([(r0, c0), (r1, c1), (r2, c2), (r0, c0),
                                 (r4, c4), (r2, c2), (r6, c6), (r4, c4)]):
        M[:, i, :] = r
        c[:, i] = cc
    return M, c


def _precompute():
    p64 = _compute_p(np.arange(-W, T_PAD + L, dtype=np.int64))
    M, c = _build_Mc(p64)
    Mb = M.reshape(N_CH_EXT, L, 8, 8)
    cb = c.reshape(N_CH_EXT, L, 8)

    Phi = np.empty((N_CH_EXT, L, 8, 8))
    Phi[:, 0] = Mb[:, 0]
    for r in range(1, L):
        Phi[:, r] = Mb[:, r] @ Phi[:, r - 1]

    K = np.zeros((N_CH_EXT, L, L))
    G = np.zeros((N_CH_EXT, 8, L))
    Tcur = cb.copy()
    for lag in range(L):
        qmax = L - lag
        idx = np.arange(qmax)
        K[:, idx + lag, idx] = Tcur[:, :qmax, 6]
        G[:, :, L - 1 - lag] = Tcur[:, L - 1 - lag, :]
        if lag < L - 1:
            nq = qmax - 1
            Tcur[:, :nq] = np.einsum('nqij,nqj->nqi', Mb[:, lag + 1:], Tcur[:, :nq])
    K[:, np.arange(L), np.arange(L)] += 1.0      # wet-mix identity on the diag

    U = Phi[:, :, 6, :].copy()                   # [N_CH_EXT, L, 8]
    P = Phi[:, L - 1].copy()                     # chunk propagators

    I8 = np.eye(8)
    Wh_all, XiT_all, XiD_all, LmE_all = [], [], [], []
    for k in range(N_CORES):
        Pq_all = P[k * 256:k * 256 + N_SQ * Q].reshape(N_SQ, Q, 8, 8)
        What = np.zeros((N_SQ, Q, 8, 8))
        Xi_T = np.zeros((N_SQ, Q, 8, 8))
        Xi_D = np.zeros((N_SQ, Q, Q, 8, 8))
        R = np.zeros((N_SQ, 8, 8))
        for q in range(N_SQ):
            Pq = Pq_all[q]
            V = np.zeros((Q, 8, 8)); V[0] = I8
            for m in range(1, Q):
                V[m] = Pq[m - 1] @ V[m - 1]
            Xi_T[q] = V
            for m in range(Q):
                acc = I8
                for mp in range(m - 1, -1, -1):
                    Xi_D[q, m, mp] = acc
                    acc = acc @ Pq[mp]
            acc = I8
            for m in range(Q - 1, -1, -1):
                What[q, m] = acc
                acc = acc @ Pq[m]
            R[q] = acc
        # T_q = sum_{qp<q} LmE[q, qp] @ E_qp   (core start state is zero)
        LmE = np.zeros((N_SQ, N_SQ, 8, 8))
        for q in range(1, N_SQ):
            acc = I8
            for qp in range(q - 1, -1, -1):
                LmE[q, qp] = acc
                acc = acc @ R[qp]
        Wh_all.append(What); XiT_all.append(Xi_T)
        XiD_all.append(Xi_D); LmE_all.append(LmE)

    return dict(K=K, U=U, G=G, Wh=Wh_all, XiT=XiT_all, XiD=XiD_all,
                LmE=LmE_all)


def _pack_core(coef, k, np_dt):
    base = k * 256

    # K anchor + delta stationaries: anchors at payload chunk 16*a
    anc_idx = base + Q + AI * np.arange(NA + 1)      # [17] ext chunk indices
    KUa = np.zeros((NA + 1, 128, 128))
    KUa[:, 0:L, 0:L] = coef['K'][anc_idx].transpose(0, 2, 1)     # K^T [tau,t]
    KUa[:, L:128, 0:L] = coef['U'][anc_idx].transpose(0, 2, 1)   # U^T [k,t]
    Kt = np.zeros((NA, 128, 256))
    Kt[:, :, 0:128] = KUa[:-1]
    Kt[:, :, 128:256] = KUa[1:] - KUa[:-1]
    Kt = (Kt.reshape(NA // KT, KT, 128, 256).transpose(0, 2, 1, 3)
          .reshape(NA // KT, 128, KT * 256))

    # pc-G anchors: one G per 4 chunks (mid-chunk), [G_a^T] blocks
    g_idx = np.minimum(base + 4 * np.arange(NG) + 2, N_CH_EXT - 1)
    Gt = coef['G'][g_idx].transpose(2, 0, 1).reshape(L, NG * 8)

    Wh = coef['Wh'][k].transpose(1, 3, 0, 2).reshape(Q * 8, N_SQ * 8)
    # all-pairs superchunk composition, payload superchunks on the out side
    LmE = (coef['LmE'][k][1:].transpose(3, 1, 0, 2)
           .reshape(8, N_SQ * 128))
    # Xi for payload superchunks only (q=1..16)
    XiT = coef['XiT'][k][1:].transpose(3, 0, 1, 2).reshape(8, ND * Q * 8)
    XiD = (coef['XiD'][k][1:].transpose(2, 4, 0, 1, 3)
           .reshape(Q * 8, ND * Q * 8))
    # alpha mask: chunk c (of 16 per group) -> alpha = c/16, all rows
    al = np.repeat(np.arange(GD) / AI, B)[None, :]               # [1, 1024]
    amask = np.broadcast_to(al, (128, GD * B)).copy()
    out = dict(Kt=Kt, Gt=Gt, Wh=Wh, LmE=LmE, XiT=XiT, XiD=XiD, amask=amask)
    return {n: np.ascontiguousarray(a.astype(np_dt)) for n, a in out.items()}


# ---------------------------------------------------------------- device
def _build_nc(mode):
    f32 = mybir.dt.float32
    dt = f32 if mode == "f32" else mybir.dt.bfloat16

    nc = bacc.Bacc(num_devices=N_CORES)
    P_ = lambda name, shape: nc.declare_dram_parameter(name, list(shape), dt,
                                                       isOutput=False)
    xT16 = P_("xT16", (L, N_SQ * GD * B))
    Kt = P_("Kt", (NA // KT, 128, KT * 256))
    Gt = P_("Gt", (L, NG * 8))
    Wh = P_("Wh", (Q * 8, N_SQ * 8))
    LmE = P_("LmE", (8, N_SQ * 128))
    XiT = P_("XiT", (8, ND * Q * 8))
    XiD = P_("XiD", (Q * 8, ND * Q * 8))
    amask = P_("amask", (128, GD * B))
    yT16 = nc.declare_dram_parameter("yT16", [ND, L, GD * B], dt, isOutput=True)

    with TileContext(nc) as tc:
        with (
            tc.tile_pool(name="const", bufs=1) as cp,
            tc.tile_pool(name="xres", bufs=1) as xpool,
            tc.tile_pool(name="kst", bufs=1) as kp,
            tc.tile_pool(name="yst", bufs=4) as yp,
            tc.tile_pool(name="xsp", bufs=1) as xsp,
            tc.tile_pool(name="svp", bufs=2) as svp,
            tc.tile_pool(name="ps_y", bufs=4, space="PSUM") as ps_y,
            tc.tile_pool(name="ps_s", bufs=2, space="PSUM") as ps_s,
            tc.tile_pool(name="ps_a", bufs=2, space="PSUM") as ps_a,
            tc.tile_pool(name="dram", bufs=1, space="DRAM") as dp,
        ):
            # ---- loads. x tiles alternate the two HWDGE rings (sync even /
            # scalar odd), K tiles interleave with the x-odd stream so both
            # arrive in consumption order; small consts go first.
            gt_t = cp.tile([L, NG * 8], dt, tag="gt")
            nc.sync.dma_start(out=gt_t[:], in_=Gt[:, :])

            def sload(param, shape, tag):
                t = cp.tile(list(shape), dt, tag=tag)
                nc.scalar.dma_start(out=t[:], in_=param[:, :])
                return t

            wh_t = sload(Wh, (Q * 8, N_SQ * 8), "wh")
            lme_t = sload(LmE, (8, N_SQ * 128), "lme")
            xit_t = sload(XiT, (8, ND * Q * 8), "xit")
            xid_t = sload(XiD, (Q * 8, ND * Q * 8), "xid")
            am_t = sload(amask, (128, GD * B), "am")
            am2_t = cp.tile([128, 2 * GD * B], dt, tag="am2")
            nc.vector.tensor_copy(out=am2_t[:, 0:GD * B], in_=am_t[:])
            nc.vector.tensor_copy(out=am2_t[:, GD * B:], in_=am_t[:])

            # x loads batched: one DMA per restack batch (rows 0:120);
            # state rows 120:128 are injected later per superchunk
            xgb = [None] * NBATCH
            kg = [None] * (NA // KT)
            for s in range(NBATCH):
                n = min(SB, N_SQ - s * SB)
                t = xpool.tile([128, n * GD * B], dt, tag=f"x{s}")
                nc.sync.dma_start(
                    out=t[0:L, :],
                    in_=xT16[:, s * SB * GD * B:(s * SB + n) * GD * B])
                xgb[s] = t
                if s < NA // KT:
                    kt = kp.tile([128, KT * 256], dt, tag=f"k{s}")
                    nc.scalar.dma_start(out=kt[:], in_=Kt[s, :, :])
                    kg[s] = kt

            def xv(q):
                """(batch tile, col offset) for superchunk q's 1024 cols."""
                return xgb[q // SB], (q % SB) * GD * B

            # ---- phase A: d_j = G_a X_j (pc-G anchors, 4 chunks/anchor);
            # a pure matmul stream -- restack bounces ride the DMA rings
            # underneath, E/Tvec/Svec matmuls come after so the PE never
            # waits mid-stream on a bounce
            dq4 = [None] * NBATCH
            e_parts = [None] * N_SQ
            for s in range(NBATCH):
                qs = list(range(s * SB, min((s + 1) * SB, N_SQ)))
                dsb = svp.tile([8, len(qs) * Q * B], dt, tag=f"dsb{s}")
                for qi, q in enumerate(qs):
                    xt, xo = xv(q)
                    for hh in range(2):
                        pda = ps_a.tile([8, 8 * B], f32, tag="pa")
                        for h2 in range(2):
                            aa = q * 4 + hh * 2 + h2
                            nc.tensor.matmul(
                                pda[:, h2 * 4 * B:(h2 + 1) * 4 * B],
                                gt_t[:, aa * 8:(aa + 1) * 8],
                                xt[0:L, xo + (hh * 2 + h2) * 4 * B:
                                   xo + (hh * 2 + h2 + 1) * 4 * B],
                                start=True, stop=True)
                        dst = dsb[:, (qi * 2 + hh) * 8 * B:
                                  (qi * 2 + hh + 1) * 8 * B]
                        if hh == 0:
                            nc.vector.tensor_copy(out=dst, in_=pda[:])
                        else:
                            nc.scalar.copy(out=dst, in_=pda[:])
                # restack [8, (s m b)] -> [(m k), (s b)] via a DRAM bounce
                dqd = dp.tile([Q * 8, len(qs) * B], dt, tag=f"dqd{s}")
                nc.scalar.dma_start(
                    out=dqd[:].rearrange("(m k) (s b) -> k s m b",
                                         m=Q, s=len(qs)),
                    in_=dsb[:].rearrange("k (s m b) -> k s m b",
                                         s=len(qs), m=Q))
                dqt = cp.tile([Q * 8, len(qs) * B], dt, tag=f"dq{s}")
                nc.sync.dma_start(out=dqt[:], in_=dqd[:])
                dq4[s] = dqt

            # ---- E_q = Wh_q D_q (restacks have landed by now)
            for q in range(N_SQ):
                s, qi = divmod(q, SB)
                pe = ps_s.tile([128, B], f32, tag="ps")
                nc.tensor.matmul(pe[0:8, :], wh_t[:, q * 8:(q + 1) * 8],
                                 dq4[s][:, qi * B:(qi + 1) * B],
                                 start=True, stop=True)
                ep = cp.tile([8, B], dt, tag=f"e{q}")
                nc.vector.tensor_copy(out=ep[:], in_=pe[0:8, :])
                e_parts[q] = ep

            # ---- all-pairs Tvec: T_q = sum_qp LmE[q,qp] E_qp, bounce to
            # [8, q-major] layout for use as matmul moving operands
            ptv = ps_s.tile([128, B], f32, tag="ps")
            for qp in range(N_SQ):
                nc.tensor.matmul(ptv[:], lme_t[:, qp * 128:(qp + 1) * 128],
                                 e_parts[qp][:],
                                 start=(qp == 0), stop=(qp == N_SQ - 1))
            tvs = svp.tile([ND * 8, B], dt, tag="tvs")
            nc.vector.tensor_copy(out=tvs[:], in_=ptv[:])
            tvT_t = cp.tile([8, ND * B], dt, tag="tvT")
            tv_dram = dp.tile([ND * 8, B], dt, tag="tvd")
            nc.scalar.dma_start(out=tv_dram[:], in_=tvs[:])
            nc.sync.dma_start(
                out=tvT_t[:].rearrange("i (q l) -> i q l", q=ND),
                in_=tv_dram[:].rearrange("(q i) l -> i q l", q=ND, i=8))

            # ---- Svec_q = XiD_q @ D_q + XiT_q @ T_q ; inject into state
            # rows (paired within a batch to halve DMA count); then the
            # alpha-scaled copy for the delta stream
            xs_tiles = [None] * N_SQ
            for pair in SV_PAIRS:
                n = len(pair)
                svs = svp.tile([Q * 8, n * B], dt, tag="svs")
                for i, q in enumerate(pair):
                    s, qi = divmod(q, SB)
                    pv = ps_s.tile([128, B], f32, tag="ps")
                    nc.tensor.matmul(pv[:], xid_t[:, (q - 1) * 128:q * 128],
                                     dq4[s][:, qi * B:(qi + 1) * B],
                                     start=True, stop=False)
                    nc.tensor.matmul(pv[:], xit_t[:, (q - 1) * 128:q * 128],
                                     tvT_t[:, (q - 1) * B:q * B],
                                     start=False, stop=True)
                    nc.vector.tensor_copy(out=svs[:, i * B:(i + 1) * B],
                                          in_=pv[:])
                q0 = pair[0]
                s0, qi0 = divmod(q0, SB)
                svd = dp.tile([Q * 8, n * B], dt, tag=f"svd{q0}")
                nc.sync.dma_start(out=svd[:], in_=svs[:])
                xt, xo = xv(q0)
                nc.sync.dma_start(
                    out=xt[L:128, xo:xo + n * GD * B]
                        .rearrange("k (s c l) -> k s c l", s=n, c=GD),
                    in_=svd[:].rearrange("(c k) (s l) -> k s c l",
                                         c=GD, s=n))
                xs = xsp.tile([128, n * GD * B], dt, tag=f"xs{q0}")
                eng = nc.vector if q0 <= 8 else nc.gpsimd
                eng.tensor_tensor(out=xs[:], in0=xt[:, xo:xo + n * GD * B],
                                  in1=am2_t[:, 0:n * GD * B],
                                  op=mybir.AluOpType.mult)
                for i, q in enumerate(pair):
                    xs_tiles[q] = (xs, i * GD * B)

            # ---- phase C: Y = KU_a @ [X;s] + DKU_a @ (alpha*[X;s]);
            # one anchor per 16-chunk group, two PSUM banks per group,
            # y stores paired across groups
            yt = None
            for g in range(ND):
                q = g + 1
                xt, xo = xv(q)
                xs, xso = xs_tiles[q]
                kt = kg[g // KT]
                co = (g % KT) * 256
                if g % 2 == 0:
                    yt = yp.tile([L, 2 * GD * B], dt, tag="y")
                yo = (g % 2) * GD * B
                for h in range(2):                    # 2 PSUM banks / group
                    py = ps_y.tile([128, 8 * B], f32, tag="py")
                    nc.tensor.matmul(
                        py[:], kt[:, co:co + 128],
                        xt[:, xo + h * 8 * B:xo + (h + 1) * 8 * B],
                        start=True, stop=False)
                    nc.tensor.matmul(
                        py[:], kt[:, co + 128:co + 256],
                        xs[:, xso + h * 8 * B:xso + (h + 1) * 8 * B],
                        start=False, stop=True)
                    dst = yt[:, yo + h * 8 * B:yo + (h + 1) * 8 * B]
                    if (g * 2 + h) % 2 == 0:
                        nc.vector.tensor_copy(out=dst, in_=py[0:L, :])
                    else:
                        nc.scalar.copy(out=dst, in_=py[0:L, :])
                if g % 2 == 1:
                    seng = nc.sync if g % 4 == 1 else nc.scalar
                    seng.dma_start(
                        out=yT16[g - 1:g + 1, :, :].rearrange("s l c -> l s c"),
                        in_=yt[:].rearrange("l (s c) -> l s c", s=2))

    nc.compile()
    return nc


# ---------------------------------------------------------------- driver
_CACHE = {}


def _get_built(mode):
    if mode not in _CACHE:
        coef = _precompute()
        np_dt = np.float32 if mode == "f32" else ml_dtypes.bfloat16
        packed = [_pack_core(coef, k, np_dt) for k in range(N_CORES)]
        nc = _build_nc(mode)
        _CACHE[mode] = (nc, packed, np_dt)
    return _CACHE[mode]


def _run(x, mode, trace=False):
    nc, packed, np_dt = _get_built(mode)
    xp = np.zeros((B, W + T_PAD), np.float32)
    xp[:, W:W + T] = np.asarray(x, dtype=np.float32)
    in_maps = []
    for k in range(N_CORES):
        xc = xp[:, k * T_C:k * T_C + W + T_C].T             # [32640, 64]
        xT16 = (xc.reshape(N_SQ, GD, L, B).transpose(2, 0, 1, 3)
                .reshape(L, N_SQ * GD * B))
        m = dict(packed[k])
        m["xT16"] = np.ascontiguousarray(xT16.astype(np_dt))
        in_maps.append(m)
    res = run_bass_kernel_spmd(nc, in_maps, list(range(N_CORES)), trace=trace)
    y = np.empty((B, T_PAD), np.float32)
    for k in range(N_CORES):
        yT16 = np.asarray(res.results[k]["yT16"]).astype(np.float32)
        yc = yT16.reshape(ND, L, GD, B).transpose(0, 2, 1, 3).reshape(T_C, B)
        y[:, k * T_C:(k + 1) * T_C] = yc.T
    return y[:, :T].astype(np.float32), res


def kernel(x):
    y, _ = _run(x, MODE, trace=False)
    return y


def run_traced(x, mode=MODE):
    return _run(x, mode, trace=True)
